# revision 39
# baseline (speedup 1.0000x reference)
"""DRQConv2d (dual-region quantized conv) Trainium2 kernel.

Reference semantics (see problem statement):
  mask  = upsample8(avgpool8(x) >= 0.05)             per (b, c)
  xh    = where(mask, x, 1e-5);  xl = where(mask, 1e-5, x)
  qh    = clip(round(xh/sh), 0, 255) * sh            (uint8 fake-quant)
  ql    = clip(round(xl/sl), 0, 15) * sl             (uint4 fake-quant)
  qwh   = per-oc quant of w_high to +-127,  qwl = per-oc quant of w_low to +-7
  y     = conv3x3(qh, qwh) + conv3x3(ql, qwl)        (pad 1)

Key implementation facts:
  * 1e-5 quantizes to exactly 0 on both paths, so the masked fill is a
    multiply by the {0,1} block mask (applied via a broadcast AP, never
    materialized at full resolution).
  * Flat-window conv: the padded 58x58 image is treated as a flat array;
    each output chunk is a 464-wide contiguous window (8 padded rows).
    A 3x3 tap is a constant flat offset in {-59..59}. Pad columns compute
    garbage that is simply never evacuated from PSUM.
  * Low conv runs in fp8e4 Double-Row mode: acts 0..15 and weights +-7 are
    exact in fp8/e6m3, so two taps contract per matmul (K=256). Tap pairs
    (-59,-1), (-58,0), (-57,1) via a second activation plane whose content
    is shifted by +58; taps 57,58,59 are plain fp8 matmuls. 6 matmuls per
    chunk instead of 9, bit-exact.
  * High conv stays bf16 (acts <=255 and weights +-127 exact); its weights
    are pre-scaled per-oc by svh/svl so both convs share PSUM banks and a
    single final evacuation scale of svl.
  * Rounding is folded into the scalar engine (x*inv_s + MAGIC in one
    Identity op); DVE does one clip + one masked-subtract-multiply per conv.
    The avgpool row-reduce runs on gpsimd.

Sharding: data-parallel over batch. 32 images -> 4 per core on 8 cores,
weights replicated; outputs concatenated on host. No collectives.
"""

import numpy as np

P = 128            # channels (both in and out) == partitions
B_TOTAL = 32
N_CORES = 8
BPC = B_TOTAL // N_CORES   # images per core
H = W = 56
HP = WP = H + 2    # zero-padded geometry
NPIX = H * W       # 3136
NPADF = HP * WP    # 3364 flat padded pixels
NW = 8 * WP        # 464: window = 8 padded rows
NTAPS = 9
NCHUNK = H // 8    # 7 windows per image
MAGIC = float(np.float32(1.5 * 2 ** 23))   # fp32 round-to-nearest magic
POOL_K = 8
THRESH = 0.05

# qh tile: [guard(1) | 3364 padded pixels | guard(1)]
QH_LEN = 1 + NPADF + 1          # 3366
# ql tile: plane0 at +1 (3364 px), plane1 content shifted by +58 with the
# DoubleRow dim-1 stride of 3424; plane1 data[k] lives at addr 3367+k.
QL_PITCH = 3424                 # DR dim1 stride, %16 == 0
QL_LEN = 2786 + 2 * QL_PITCH + 16   # max slice end for the rearrange trick
# interior write offset: valid pixel (r, c) -> flat 58*(r+1) + (c+1)
QH_INT = 60                     # 58 + 1 + 1
QL1_INT = 3426                  # 3367 + 59

# tap flat offsets, high conv rhs slice starts relative to window start o:
# delta = (kh-1)*58 + (kw-1) - (-59)  ->  o + [0,1,2, 58,59,60, 116,117,118]
HIGH_TAP_OFF = [0, 1, 2, 58, 59, 60, 116, 117, 118]
# low conv DR pairs read overlapping APs out of the single activation
# plane: dim1 step 58 pairs tap delta with delta+58, step 1 pairs 57 with 58.
# (slot_a, slot_b, rhs offset rel. to window, dim1 step)
LOW_PAIRS = [(0, 3, 0, WP), (1, 4, 1, WP), (2, 5, 2, WP), (6, 7, 116, 1)]
LOW_SINGLE_SLOTS = [8]
LOW_SINGLE_OFF = [118]


def build_program(nc, tc, aps, inv_sh, inv_sl, c_svh, c_svl, bpc=BPC):
    """Emit the whole per-core program inside an open TileContext."""
    import concourse.mybir as mybir
    from concourse.alu_op_type import AluOpType as op
    from concourse.masks import make_identity

    f32 = mybir.dt.float32
    bf16 = mybir.dt.bfloat16
    fp8 = mybir.dt.float8e4
    X = mybir.AxisListType.X
    DR = mybir.MatmulPerfMode.DoubleRow
    Identity = mybir.ActivationFunctionType.Identity

    x_d, wh_d, wl_d, y_d = aps["x"], aps["w_high"], aps["w_low"], aps["y"]

    sum_thresh = float(np.float32(THRESH) * POOL_K * POOL_K)

    with (
        tc.tile_pool(name="persist", bufs=1) as persist,
        tc.tile_pool(name="wtmp", bufs=2) as wtmp_pool,
        tc.tile_pool(name="tp_psum", bufs=1, space="PSUM") as tp_psum,
        tc.tile_pool(name="xin", bufs=3) as xin,
        tc.tile_pool(name="acts", bufs=2) as acts,
        tc.tile_pool(name="masks", bufs=2) as maskp,
        tc.tile_pool(name="tree", bufs=1) as treep,
        tc.tile_pool(name="outs", bufs=4) as outs_pool,
        tc.tile_pool(name="conv_psum", bufs=7, space="PSUM") as conv_psum,
    ):
        identity = persist.tile([P, P], f32)
        make_identity(nc, identity[:])
        magic_ap = persist.tile([P, 1], f32, name="magic")
        nc.gpsimd.memset(magic_ap[:], MAGIC)

        # persistent quantized-activation tiles (double buffered). Borders /
        # pads / guards are zeroed ONCE here and never written again; the
        # per-image stt ops only overwrite interior pixels.
        qh_bufs = [persist.tile([P, QH_LEN], bf16, name=f"qhb{i}")
                   for i in range(2)]
        ql_bufs = [persist.tile([P, QH_LEN], fp8, name=f"qlb{i}")
                   for i in range(2)]

        def zero_pads(t, base):
            # zero only the pad/guard positions of one 58x58 plane at `base`:
            # head (guard + row0 + row1col0), interior col-pad pairs, tail.
            nc.vector.memset(t[:, base:base + 60], 0.0)
            nc.vector.memset(
                t[:, base + 116:base + 116 + 55 * WP]
                .rearrange("p (r c) -> p r c", r=55)[:, :, 0:2],
                0.0,
            )
            nc.vector.memset(t[:, base + 3306:base + 3366], 0.0)

        for t in qh_bufs + ql_bufs:
            zero_pads(t, 0)

        # rounded-activation tiles with one guard col each side (the masked
        # store reads them with a -1 col shift); guards zeroed once so the
        # first image can't read a NaN bit pattern.
        t_bufs = [[persist.tile([P, NPIX + 2], f32, name=f"tb{i}{j}")
                   for j in range(2)] for i in range(2)]
        for pair in t_bufs:
            for t in pair:
                # guards + the nsplit=2 boundary element (never written
                # before the first image's first-half store reads them)
                nc.vector.memset(t[:, 0:1], 0.0)
                nc.vector.memset(t[:, 28 * W + 1:28 * W + 2], 0.0)
                nc.vector.memset(t[:, NPIX + 1:NPIX + 2], 0.0)

        # ---------------- weight prep ----------------
        # low first (its scale feeds the high-weight ratio)
        wq = {}
        sv = {}

        def weight_quant(conv, w_dram, nw, c_sv):
            wnat = wtmp_pool.tile([P, P * NTAPS], f32, tag="wnat")
            nc.sync.dma_start(out=wnat[:], in_=w_dram)
            absmax = persist.tile([P, 1], f32, name=f"absmax_{conv}")
            nc.vector.tensor_reduce(
                absmax[:], wnat[:], axis=X, op=op.max, apply_absolute_value=True
            )
            sv_t = persist.tile([P, 1], f32, name=f"sv_{conv}")
            nc.vector.tensor_scalar_mul(sv_t[:], absmax[:], c_sv)
            sv[conv] = sv_t
            rcp = persist.tile([P, 1], f32, name=f"rcp_{conv}")
            nc.vector.reciprocal(rcp[:], absmax[:])
            rs = persist.tile([P, 1], f32, name=f"rs_{conv}")
            nc.vector.tensor_scalar_mul(rs[:], rcp[:], nw)
            # integer-quantize in natural [oc, ic*9] layout
            wqt = wtmp_pool.tile([P, P * NTAPS], f32, tag=f"wq_{conv}")
            nc.vector.tensor_scalar(
                wqt[:], wnat[:], rs[:, 0:1], MAGIC, op0=op.mult, op1=op.add
            )
            nc.vector.tensor_scalar(
                wqt[:], wqt[:], MAGIC, nw, op0=op.subtract, op1=op.min
            )
            nc.vector.tensor_scalar_max(wqt[:], wqt[:], -nw)
            wq[conv] = wqt

        weight_quant("l", wl_d, 7.0, c_svl)
        weight_quant("h", wh_d, 127.0, c_svh)

        # scale high weights by svh/svl (per-oc) so both convs share PSUM
        rcp_svl = persist.tile([P, 1], f32, name="rcp_svl")
        nc.vector.reciprocal(rcp_svl[:], sv["l"][:, 0:1])
        ratio_h = persist.tile([P, 1], f32, name="ratio_h")
        nc.vector.tensor_tensor(
            ratio_h[:], sv["h"][:, 0:1], rcp_svl[:], op=op.mult
        )
        nc.vector.tensor_scalar_mul(wq["h"][:], wq["h"][:], ratio_h[:, 0:1])

        # transpose taps: [oc, ic] -> [ic, oc] via PE, evacuate with dtype cast
        qwt_h = persist.tile([P, NTAPS * P], bf16, name="qwt_h")
        qwl_pairs = persist.tile([P, 8 * P], fp8, name="qwl_pairs")
        qwl_single = persist.tile([P, 1 * P], fp8, name="qwl_single")

        def transpose_taps(conv, dst_list):
            # dst_list: per-tap destination AP ([P, P] slices)
            wq_v = wq[conv][:].rearrange("p (i t) -> p t i", t=NTAPS)
            for base in range(0, NTAPS, 4):
                n = min(4, NTAPS - base)
                tp = tp_psum.tile([P, 4 * P], f32, tag="tp")
                for j in range(n):
                    nc.tensor.transpose(
                        tp[:, j * P:(j + 1) * P],
                        wq_v[:, base + j, :], identity[:],
                    )
                for j in range(n):
                    nc.vector.tensor_copy(
                        out=dst_list[base + j], in_=tp[:, j * P:(j + 1) * P]
                    )

        transpose_taps("h", [qwt_h[:, t * P:(t + 1) * P] for t in range(NTAPS)])
        low_dst = [None] * NTAPS
        for i, (ta, tb, _, _) in enumerate(LOW_PAIRS):
            low_dst[ta] = qwl_pairs[:, (2 * i) * P:(2 * i + 1) * P]
            low_dst[tb] = qwl_pairs[:, (2 * i + 1) * P:(2 * i + 2) * P]
        for s, t in enumerate(LOW_SINGLE_SLOTS):
            low_dst[t] = qwl_single[:, s * P:(s + 1) * P]
        transpose_taps("l", low_dst)

        # PE warm-up: HAM un-throttles after ~3.4us of sustained activity.
        warm_ps = tp_psum.tile([P, 4 * P], f32, tag="tp")
        for i in range(28):
            nc.tensor.matmul(
                warm_ps[:, 0:P], identity[:], identity[:],
                start=(i == 0), stop=(i == 27),
            )

        # ---------------- per-image ops ----------------
        xts = {}

        def fetch(b):
            if b < bpc and b not in xts:
                xts[b] = xin.tile([P, NPIX], f32, tag="xt", name=f"xt{b}")
                nc.sync.dma_start(out=xts[b][:], in_=x_d[b])

        def prep(b, nsplit=1):
            """Mask + quantized activations for image b.

            DVE is by far the fastest elementwise engine (~0.2 ns/col vs
            ~0.83 for ACT/gpsimd), so it keeps the big masked stores; the
            avgpool tree and the clamps run on the otherwise-idle gpsimd,
            ACT rounds (x*inv_s + MAGIC) and evacuates PSUM.
            nsplit=2 emits the quant chain in two row-halves so image 0's
            first chunks can start before the whole image is quantized."""
            xt = xts.pop(b)
            # --- mask: blocksums -> threshold -> full masks ---
            # image 0 reduces on DVE (gpsimd is busy quantizing weights);
            # later images use a gpsimd pairwise tree to keep DVE free.
            r1 = maskp.tile([P, H * NCHUNK], f32, tag="r1")   # (wb, h) order
            if True:
                nc.vector.tensor_reduce(
                    r1[:].rearrange("p (w h) -> p h w", w=NCHUNK),
                    xt[:].rearrange("p (r c) -> p r c", c=POOL_K),
                    axis=X, op=op.add,
                )
            elif False:
                s1 = treep.tile([P, NPIX // 2], f32, tag="s1")
                v0 = xt[:].rearrange("p (n two) -> p n two", two=2)
                nc.gpsimd.tensor_add(s1[:], v0[:, :, 0], v0[:, :, 1])
                s2 = treep.tile([P, NPIX // 4], f32, tag="s2")
                v1 = s1[:].rearrange("p (n two) -> p n two", two=2)
                nc.gpsimd.tensor_add(s2[:], v1[:, :, 0], v1[:, :, 1])
                v2 = s2[:].rearrange("p (h w two) -> p h w two", h=H, two=2)
                nc.gpsimd.tensor_add(
                    r1[:].rearrange("p (w h) -> p h w", w=NCHUNK),
                    v2[:, :, :, 0], v2[:, :, :, 1],
                )
            r2 = maskp.tile([P, NCHUNK * NCHUNK], f32, tag="r2")
            nc.vector.tensor_reduce(
                r2[:], r1[:].rearrange("p (g c) -> p g c", c=POOL_K),
                axis=X, op=op.add,
            )
            mt = maskp.tile([P, NCHUNK * NCHUNK], f32, tag="mt")
            nc.vector.tensor_scalar(
                mt[:], r2[:], sum_thresh, None, op0=op.is_ge
            )
            m49h = maskp.tile([P, NCHUNK * NCHUNK], f32, tag="m49h")
            nc.vector.tensor_copy(
                out=m49h[:], in_=mt[:].rearrange("p (w h) -> p h w", w=NCHUNK)
            )
            m49l = maskp.tile([P, NCHUNK * NCHUNK], f32, tag="m49l")
            nc.vector.tensor_scalar(
                m49l[:], m49h[:], -1.0, 1.0, op0=op.mult, op1=op.add
            )

            # 58-wide row-masks and full-res masks with ZERO pad columns, so
            # the masked stores can write fully-contiguous 58-wide rows (the
            # pad columns compute 0 via the mask).
            cast_eng = nc.vector

            def expand_mask(m49, tagsuffix):
                mr = maskp.tile([P, NCHUNK * W], f32, tag=f"mr{tagsuffix}")
                nc.vector.tensor_copy(
                    out=mr[:].rearrange("p (g c) -> p g c", c=POOL_K),
                    in_=m49[:].unsqueeze(2).broadcast_to((P, 49, POOL_K)),
                )
                mr58 = maskp.tile([P, NCHUNK * WP], f32, tag=f"mr58{tagsuffix}")
                nc.vector.memset(
                    mr58[:].rearrange("p (g c) -> p g c", c=WP)
                    [:, :, 0:WP:WP - 1], 0.0,
                )
                nc.vector.tensor_copy(
                    out=mr58[:].rearrange("p (g c) -> p g c", c=WP)[:, :, 1:57],
                    in_=mr[:].rearrange("p (g c) -> p g c", c=W),
                )
                mexp = maskp.tile([P, H * WP], fp8, tag=f"mexp{tagsuffix}")
                me3 = mexp[:].rearrange("p (r c) -> p r c", r=H)
                for hb in range(NCHUNK):
                    # image 0's casts go to ACT: the startup critical path
                    # is the serial DVE chain, and ACT is idle then
                    if cast_eng is nc.scalar:
                        nc.scalar.copy(
                            me3[:, hb * POOL_K:(hb + 1) * POOL_K, :],
                            mr58[:, hb * WP:(hb + 1) * WP]
                            .unsqueeze(1).broadcast_to((P, POOL_K, WP)),
                        )
                    else:
                        nc.vector.tensor_copy(
                            out=me3[:, hb * POOL_K:(hb + 1) * POOL_K, :],
                            in_=mr58[:, hb * WP:(hb + 1) * WP]
                            .unsqueeze(1).broadcast_to((P, POOL_K, WP)),
                        )
                return mexp

            mexp_h = expand_mask(m49h, "h")
            mexp_l = expand_mask(m49l, "l")

            # --- quantize: ACT rounds, DVE clamps + masked-stores ---
            # t tiles have one guard col each side; the store reads t at a
            # -1 column shift so its 58-wide rows stay contiguous.
            qh = qh_bufs[b % 2]
            ql = ql_bufs[b % 2]
            th = t_bufs[b % 2][0]
            tl = t_bufs[b % 2][1]
            bounds = [(i * H) // nsplit for i in range(nsplit + 1)]
            for ra, rb in zip(bounds, bounds[1:]):
                sl_ = slice(1 + ra * W, 1 + rb * W)
                for t, inv_s, qmax in ((th, inv_sh, 255.0), (tl, inv_sl, 15.0)):
                    nc.scalar.activation(
                        t[:, sl_], xt[:, ra * W:rb * W], Identity,
                        bias=magic_ap[:, 0:1], scale=inv_s,
                    )
                    nc.vector.tensor_scalar(
                        t[:, sl_], t[:, sl_], MAGIC, MAGIC + qmax,
                        op0=op.max, op1=op.min,
                    )
                nrows = rb - ra
                for t, mexp, tile_ap in ((th, mexp_h, qh), (tl, mexp_l, ql)):
                    # in0 rows overlap: 58 cols at row-stride 56 reads the
                    # pixel at (r, c-1); pad cols read neighbors * mask 0.
                    in0 = (
                        t[:, ra * W:rb * W]
                        .rearrange("p (r c) -> p r c", r=nrows)
                    )
                    in0.ap[2] = [1, WP]
                    nc.vector.scalar_tensor_tensor(
                        out=tile_ap[:, 59 + ra * WP:59 + rb * WP]
                        .rearrange("p (r c) -> p r c", r=nrows),
                        in0=in0,
                        scalar=MAGIC,
                        in1=mexp[:, ra * WP:rb * WP]
                        .rearrange("p (r c) -> p r c", r=nrows),
                        op0=op.subtract, op1=op.mult,
                    )
            return qh, ql

        def conv_chunk(b, qa, c):
            """One 464-wide window: 9 bf16 high taps + 3 DR + 3 single fp8
            low taps, all accumulating into one PSUM bank."""
            qh, ql = qa
            ps = conv_psum.tile([P, NW], f32, tag="ps", name=f"ps{b}_{c}")
            o = NW * c
            for t in range(NTAPS):
                nc.tensor.matmul(
                    ps[:], qwt_h[:, t * P:(t + 1) * P],
                    qh[:, o + HIGH_TAP_OFF[t]:o + HIGH_TAP_OFF[t] + NW],
                    start=(t == 0), stop=False,
                )
            for i, (_, _, roff, step) in enumerate(LOW_PAIRS):
                o0 = o + roff
                # overlapping AP: dim1 step pairs the window at taps
                # (delta, delta+step) out of the single activation plane
                rhs = (
                    ql[:, o0:o0 + NW].unsqueeze(1).broadcast_to((P, 2, NW))
                )
                rhs.ap[1] = [step, 2]
                lhsT = (
                    qwl_pairs[:, 2 * i * P:(2 * i + 2) * P]
                    .rearrange("p (two m) -> p two m", two=2)
                )
                nc.tensor.matmul(
                    ps[:], lhsT, rhs, start=False, stop=False, perf_mode=DR,
                )
            for s, soff in enumerate(LOW_SINGLE_OFF):
                o0 = o + soff
                nc.tensor.matmul(
                    ps[:], qwl_single[:, s * P:(s + 1) * P],
                    ql[:, o0:o0 + NW],
                    start=False, stop=(s == len(LOW_SINGLE_OFF) - 1),
                )
            return ps

        def evac_chunk(b, ps, c):
            seg = outs_pool.tile([P, H * POOL_K], bf16, tag="seg")
            psv = ps[:].rearrange("p (r c) -> p r c", r=POOL_K)[:, :, 1:1 + W]
            nc.scalar.mul(seg[:], psv, sv["l"][:, 0:1])
            nc.sync.dma_start(
                out=y_d[b][:, c * H * POOL_K:(c + 1) * H * POOL_K], in_=seg[:]
            )

        # ---------------- schedule ----------------
        # software pipeline, two images deep: prep(b) for images 0 and 1 is
        # emitted up-front; during image b's convs we emit prep(b+2), whose
        # stores wait (WAR) on convs(b) releasing the shared buffer but
        # queue early so each engine's FIFO drains without head-of-line
        # blocking.
        fetch(0)
        fetch(1)
        qa = prep(0, nsplit=2)
        for b in range(bpc):
            fetch(b + 2)
            pss = []
            for c in range(NCHUNK):
                pss.append(conv_chunk(b, qa, c))
                if c < 3:
                    evac_chunk(b, pss[c], c)
            if b + 1 < bpc:
                qa = prep(b + 1)
            for c in range(3, NCHUNK):
                evac_chunk(b, pss[c], c)


def make_bass(inv_sh, inv_sl, c_svh, c_svl, bpc=BPC):
    import concourse.bacc as bacc
    import concourse.mybir as mybir
    from concourse.tile import TileContext

    f32 = mybir.dt.float32
    nc = bacc.Bacc("TRN2", debug=False)
    x = nc.dram_tensor("x", [bpc, P, NPIX], f32, kind="ExternalInput")
    wh = nc.dram_tensor("w_high", [P, P * NTAPS], f32, kind="ExternalInput")
    wl = nc.dram_tensor("w_low", [P, P * NTAPS], f32, kind="ExternalInput")
    y = nc.dram_tensor(
        "y", [bpc, P, NPIX], mybir.dt.bfloat16, kind="ExternalOutput"
    )
    aps = {"x": x.ap(), "w_high": wh.ap(), "w_low": wl.ap(), "y": y.ap()}
    with TileContext(nc) as tc:
        build_program(nc, tc, aps, inv_sh, inv_sl, c_svh, c_svl, bpc=bpc)
    nc.compile()
    return nc


def _scale_consts(act_scale_high, act_scale_low):
    sh = float(np.float32(act_scale_high))
    sl = float(np.float32(act_scale_low))
    inv_sh = float(np.float32(1.0 / np.float64(sh)))
    inv_sl = float(np.float32(1.0 / np.float64(sl)))
    c_svh = float(np.float32(np.float64(sh) / 127.0))
    c_svl = float(np.float32(np.float64(sl) / 7.0))
    return inv_sh, inv_sl, c_svh, c_svl


def _run(x, w_high, w_low, act_scale_high, act_scale_low, trace=False, **kw):
    from concourse import bass_utils

    x = np.ascontiguousarray(np.asarray(x, dtype=np.float32))
    w_high = np.ascontiguousarray(np.asarray(w_high, dtype=np.float32))
    w_low = np.ascontiguousarray(np.asarray(w_low, dtype=np.float32))

    inv_sh, inv_sl, c_svh, c_svl = _scale_consts(act_scale_high, act_scale_low)
    nc = make_bass(inv_sh, inv_sl, c_svh, c_svl)

    wh_flat = w_high.reshape(P, P * NTAPS)
    wl_flat = w_low.reshape(P, P * NTAPS)
    in_maps = []
    for core in range(N_CORES):
        xs = x[core * BPC:(core + 1) * BPC].reshape(BPC, P, NPIX)
        in_maps.append(
            {
                "x": np.ascontiguousarray(xs),
                "w_high": wh_flat,
                "w_low": wl_flat,
            }
        )
    res = bass_utils.run_bass_kernel_spmd(
        nc, in_maps, core_ids=list(range(N_CORES)), trace=trace, **kw
    )
    y = np.concatenate(
        [np.asarray(r["y"]).astype(np.float32).reshape(BPC, P, H, W)
         for r in res.results], axis=0)
    return y, res


def kernel(x, w_high, w_low, act_scale_high, act_scale_low):
    y, _ = _run(x, w_high, w_low, act_scale_high, act_scale_low)
    return y


# revision 40
# speedup vs baseline: 1.0052x; 1.0052x over previous
"""DRQConv2d (dual-region quantized conv) Trainium2 kernel.

Reference semantics (see problem statement):
  mask  = upsample8(avgpool8(x) >= 0.05)             per (b, c)
  xh    = where(mask, x, 1e-5);  xl = where(mask, 1e-5, x)
  qh    = clip(round(xh/sh), 0, 255) * sh            (uint8 fake-quant)
  ql    = clip(round(xl/sl), 0, 15) * sl             (uint4 fake-quant)
  qwh   = per-oc quant of w_high to +-127,  qwl = per-oc quant of w_low to +-7
  y     = conv3x3(qh, qwh) + conv3x3(ql, qwl)        (pad 1)

Key implementation facts:
  * 1e-5 quantizes to exactly 0 on both paths, so the masked fill is a
    multiply by the {0,1} block mask (applied via a broadcast AP, never
    materialized at full resolution).
  * Flat-window conv: the padded 58x58 image is treated as a flat array;
    each output chunk is a 464-wide contiguous window (8 padded rows).
    A 3x3 tap is a constant flat offset in {-59..59}. Pad columns compute
    garbage that is simply never evacuated from PSUM.
  * Low conv runs in fp8e4 Double-Row mode: acts 0..15 and weights +-7 are
    exact in fp8/e6m3, so two taps contract per matmul (K=256). Tap pairs
    (-59,-1), (-58,0), (-57,1) via a second activation plane whose content
    is shifted by +58; taps 57,58,59 are plain fp8 matmuls. 6 matmuls per
    chunk instead of 9, bit-exact.
  * High conv stays bf16 (acts <=255 and weights +-127 exact); its weights
    are pre-scaled per-oc by svh/svl so both convs share PSUM banks and a
    single final evacuation scale of svl.
  * Rounding is folded into the scalar engine (x*inv_s + MAGIC in one
    Identity op); DVE does one clip + one masked-subtract-multiply per conv.
    The avgpool row-reduce runs on gpsimd.

Sharding: data-parallel over batch. 32 images -> 4 per core on 8 cores,
weights replicated; outputs concatenated on host. No collectives.
"""

import numpy as np

P = 128            # channels (both in and out) == partitions
B_TOTAL = 32
N_CORES = 8
BPC = B_TOTAL // N_CORES   # images per core
H = W = 56
HP = WP = H + 2    # zero-padded geometry
NPIX = H * W       # 3136
NPADF = HP * WP    # 3364 flat padded pixels
NW = 8 * WP        # 464: window = 8 padded rows
NTAPS = 9
NCHUNK = H // 8    # 7 windows per image
MAGIC = float(np.float32(1.5 * 2 ** 23))   # fp32 round-to-nearest magic
POOL_K = 8
THRESH = 0.05

# qh tile: [guard(1) | 3364 padded pixels | guard(1)]
QH_LEN = 1 + NPADF + 1          # 3366
# ql tile: plane0 at +1 (3364 px), plane1 content shifted by +58 with the
# DoubleRow dim-1 stride of 3424; plane1 data[k] lives at addr 3367+k.
QL_PITCH = 3424                 # DR dim1 stride, %16 == 0
QL_LEN = 2786 + 2 * QL_PITCH + 16   # max slice end for the rearrange trick
# interior write offset: valid pixel (r, c) -> flat 58*(r+1) + (c+1)
QH_INT = 60                     # 58 + 1 + 1
QL1_INT = 3426                  # 3367 + 59

# tap flat offsets, high conv rhs slice starts relative to window start o:
# delta = (kh-1)*58 + (kw-1) - (-59)  ->  o + [0,1,2, 58,59,60, 116,117,118]
HIGH_TAP_OFF = [0, 1, 2, 58, 59, 60, 116, 117, 118]
# low conv DR pairs read overlapping APs out of the single activation
# plane: dim1 step 58 pairs tap delta with delta+58, step 1 pairs 57 with 58.
# (slot_a, slot_b, rhs offset rel. to window, dim1 step)
LOW_PAIRS = [(0, 3, 0, WP), (1, 4, 1, WP), (2, 5, 2, WP), (6, 7, 116, 1)]
LOW_SINGLE_SLOTS = [8]
LOW_SINGLE_OFF = [118]


def build_program(nc, tc, aps, inv_sh, inv_sl, c_svh, c_svl, bpc=BPC):
    """Emit the whole per-core program inside an open TileContext."""
    import concourse.mybir as mybir
    from concourse.alu_op_type import AluOpType as op
    from concourse.masks import make_identity

    f32 = mybir.dt.float32
    bf16 = mybir.dt.bfloat16
    fp8 = mybir.dt.float8e4
    X = mybir.AxisListType.X
    DR = mybir.MatmulPerfMode.DoubleRow
    Identity = mybir.ActivationFunctionType.Identity

    x_d, wh_d, wl_d, y_d = aps["x"], aps["w_high"], aps["w_low"], aps["y"]

    sum_thresh = float(np.float32(THRESH) * POOL_K * POOL_K)

    with (
        tc.tile_pool(name="persist", bufs=1) as persist,
        tc.tile_pool(name="wtmp", bufs=2) as wtmp_pool,
        tc.tile_pool(name="tp_psum", bufs=1, space="PSUM") as tp_psum,
        tc.tile_pool(name="xin", bufs=3) as xin,
        tc.tile_pool(name="acts", bufs=2) as acts,
        tc.tile_pool(name="masks", bufs=2) as maskp,
        tc.tile_pool(name="tree", bufs=1) as treep,
        tc.tile_pool(name="outs", bufs=4) as outs_pool,
        tc.tile_pool(name="conv_psum", bufs=7, space="PSUM") as conv_psum,
    ):
        identity = persist.tile([P, P], f32)
        make_identity(nc, identity[:])
        magic_ap = persist.tile([P, 1], f32, name="magic")
        nc.gpsimd.memset(magic_ap[:], MAGIC)

        # persistent quantized-activation tiles (double buffered). Borders /
        # pads / guards are zeroed ONCE here and never written again; the
        # per-image stt ops only overwrite interior pixels.
        qh_bufs = [persist.tile([P, QH_LEN], bf16, name=f"qhb{i}")
                   for i in range(2)]
        ql_bufs = [persist.tile([P, QH_LEN], fp8, name=f"qlb{i}")
                   for i in range(2)]

        def zero_pads(t, base):
            # zero only the pad/guard positions of one 58x58 plane at `base`:
            # head (guard + row0 + row1col0), interior col-pad pairs, tail.
            nc.vector.memset(t[:, base:base + 60], 0.0)
            nc.vector.memset(
                t[:, base + 116:base + 116 + 55 * WP]
                .rearrange("p (r c) -> p r c", r=55)[:, :, 0:2],
                0.0,
            )
            nc.vector.memset(t[:, base + 3306:base + 3366], 0.0)

        for t in qh_bufs + ql_bufs:
            zero_pads(t, 0)

        # rounded-activation tiles with one guard col each side (the masked
        # store reads them with a -1 col shift); guards zeroed once so the
        # first image can't read a NaN bit pattern.
        t_bufs = [[persist.tile([P, NPIX + 2], f32, name=f"tb{i}{j}")
                   for j in range(2)] for i in range(2)]
        for pair in t_bufs:
            for t in pair:
                # guards + the nsplit=2 boundary element (never written
                # before the first image's first-half store reads them)
                nc.vector.memset(t[:, 0:1], 0.0)
                nc.vector.memset(t[:, 28 * W + 1:28 * W + 2], 0.0)
                nc.vector.memset(t[:, NPIX + 1:NPIX + 2], 0.0)

        # ---------------- weight prep ----------------
        # low first (its scale feeds the high-weight ratio)
        wq = {}
        sv = {}

        def weight_quant(conv, w_dram, nw, c_sv):
            wnat = wtmp_pool.tile([P, P * NTAPS], f32, tag="wnat")
            nc.sync.dma_start(out=wnat[:], in_=w_dram)
            absmax = persist.tile([P, 1], f32, name=f"absmax_{conv}")
            nc.vector.tensor_reduce(
                absmax[:], wnat[:], axis=X, op=op.max, apply_absolute_value=True
            )
            sv_t = persist.tile([P, 1], f32, name=f"sv_{conv}")
            nc.vector.tensor_scalar_mul(sv_t[:], absmax[:], c_sv)
            sv[conv] = sv_t
            rcp = persist.tile([P, 1], f32, name=f"rcp_{conv}")
            nc.vector.reciprocal(rcp[:], absmax[:])
            rs = persist.tile([P, 1], f32, name=f"rs_{conv}")
            nc.vector.tensor_scalar_mul(rs[:], rcp[:], nw)
            # integer-quantize in natural [oc, ic*9] layout
            wqt = wtmp_pool.tile([P, P * NTAPS], f32, tag=f"wq_{conv}")
            nc.vector.tensor_scalar(
                wqt[:], wnat[:], rs[:, 0:1], MAGIC, op0=op.mult, op1=op.add
            )
            nc.vector.tensor_scalar(
                wqt[:], wqt[:], MAGIC, nw, op0=op.subtract, op1=op.min
            )
            nc.vector.tensor_scalar_max(wqt[:], wqt[:], -nw)
            wq[conv] = wqt

        weight_quant("l", wl_d, 7.0, c_svl)
        weight_quant("h", wh_d, 127.0, c_svh)

        # scale high weights by svh/svl (per-oc) so both convs share PSUM
        rcp_svl = persist.tile([P, 1], f32, name="rcp_svl")
        nc.vector.reciprocal(rcp_svl[:], sv["l"][:, 0:1])
        ratio_h = persist.tile([P, 1], f32, name="ratio_h")
        nc.vector.tensor_tensor(
            ratio_h[:], sv["h"][:, 0:1], rcp_svl[:], op=op.mult
        )
        nc.vector.tensor_scalar_mul(wq["h"][:], wq["h"][:], ratio_h[:, 0:1])

        # transpose taps: [oc, ic] -> [ic, oc] via PE, evacuate with dtype cast
        qwt_h = persist.tile([P, NTAPS * P], bf16, name="qwt_h")
        qwl_pairs = persist.tile([P, 8 * P], fp8, name="qwl_pairs")
        qwl_single = persist.tile([P, 1 * P], fp8, name="qwl_single")

        def transpose_taps(conv, dst_list):
            # dst_list: per-tap destination AP ([P, P] slices)
            wq_v = wq[conv][:].rearrange("p (i t) -> p t i", t=NTAPS)
            for base in range(0, NTAPS, 4):
                n = min(4, NTAPS - base)
                tp = tp_psum.tile([P, 4 * P], f32, tag="tp")
                for j in range(n):
                    nc.tensor.transpose(
                        tp[:, j * P:(j + 1) * P],
                        wq_v[:, base + j, :], identity[:],
                    )
                for j in range(n):
                    nc.vector.tensor_copy(
                        out=dst_list[base + j], in_=tp[:, j * P:(j + 1) * P]
                    )

        transpose_taps("h", [qwt_h[:, t * P:(t + 1) * P] for t in range(NTAPS)])
        low_dst = [None] * NTAPS
        for i, (ta, tb, _, _) in enumerate(LOW_PAIRS):
            low_dst[ta] = qwl_pairs[:, (2 * i) * P:(2 * i + 1) * P]
            low_dst[tb] = qwl_pairs[:, (2 * i + 1) * P:(2 * i + 2) * P]
        for s, t in enumerate(LOW_SINGLE_SLOTS):
            low_dst[t] = qwl_single[:, s * P:(s + 1) * P]
        transpose_taps("l", low_dst)

        # PE warm-up: HAM un-throttles after ~3.4us of sustained activity.
        warm_ps = tp_psum.tile([P, 4 * P], f32, tag="tp")
        for i in range(28):
            nc.tensor.matmul(
                warm_ps[:, 0:P], identity[:], identity[:],
                start=(i == 0), stop=(i == 27),
            )

        # ---------------- per-image ops ----------------
        xts = {}

        def fetch(b):
            if b < bpc and b not in xts:
                xts[b] = xin.tile([P, NPIX], f32, tag="xt", name=f"xt{b}")
                nc.sync.dma_start(out=xts[b][:], in_=x_d[b])

        def prep(b, nsplit=1):
            """Mask + quantized activations for image b.

            DVE is by far the fastest elementwise engine (~0.2 ns/col vs
            ~0.83 for ACT/gpsimd), so it keeps the big masked stores; the
            avgpool tree and the clamps run on the otherwise-idle gpsimd,
            ACT rounds (x*inv_s + MAGIC) and evacuates PSUM.
            nsplit=2 emits the quant chain in two row-halves so image 0's
            first chunks can start before the whole image is quantized."""
            xt = xts.pop(b)
            # --- mask: blocksums -> threshold -> full masks ---
            # image 0 reduces on DVE (gpsimd is busy quantizing weights);
            # later images use a gpsimd pairwise tree to keep DVE free.
            r1 = maskp.tile([P, H * NCHUNK], f32, tag="r1")   # (wb, h) order
            if True:
                nc.vector.tensor_reduce(
                    r1[:].rearrange("p (w h) -> p h w", w=NCHUNK),
                    xt[:].rearrange("p (r c) -> p r c", c=POOL_K),
                    axis=X, op=op.add,
                )
            elif False:
                s1 = treep.tile([P, NPIX // 2], f32, tag="s1")
                v0 = xt[:].rearrange("p (n two) -> p n two", two=2)
                nc.gpsimd.tensor_add(s1[:], v0[:, :, 0], v0[:, :, 1])
                s2 = treep.tile([P, NPIX // 4], f32, tag="s2")
                v1 = s1[:].rearrange("p (n two) -> p n two", two=2)
                nc.gpsimd.tensor_add(s2[:], v1[:, :, 0], v1[:, :, 1])
                v2 = s2[:].rearrange("p (h w two) -> p h w two", h=H, two=2)
                nc.gpsimd.tensor_add(
                    r1[:].rearrange("p (w h) -> p h w", w=NCHUNK),
                    v2[:, :, :, 0], v2[:, :, :, 1],
                )
            r2 = maskp.tile([P, NCHUNK * NCHUNK], f32, tag="r2")
            nc.vector.tensor_reduce(
                r2[:], r1[:].rearrange("p (g c) -> p g c", c=POOL_K),
                axis=X, op=op.add,
            )
            mt = maskp.tile([P, NCHUNK * NCHUNK], f32, tag="mt")
            nc.vector.tensor_scalar(
                mt[:], r2[:], sum_thresh, None, op0=op.is_ge
            )
            m49h = maskp.tile([P, NCHUNK * NCHUNK], f32, tag="m49h")
            nc.vector.tensor_copy(
                out=m49h[:], in_=mt[:].rearrange("p (w h) -> p h w", w=NCHUNK)
            )
            m49l = maskp.tile([P, NCHUNK * NCHUNK], f32, tag="m49l")
            nc.vector.tensor_scalar(
                m49l[:], m49h[:], -1.0, 1.0, op0=op.mult, op1=op.add
            )

            # 58-wide row-masks and full-res masks with ZERO pad columns, so
            # the masked stores can write fully-contiguous 58-wide rows (the
            # pad columns compute 0 via the mask).
            cast_eng = nc.vector

            def expand_mask(m49, tagsuffix):
                mr = maskp.tile([P, NCHUNK * W], f32, tag=f"mr{tagsuffix}")
                nc.vector.tensor_copy(
                    out=mr[:].rearrange("p (g c) -> p g c", c=POOL_K),
                    in_=m49[:].unsqueeze(2).broadcast_to((P, 49, POOL_K)),
                )
                mr58 = maskp.tile([P, NCHUNK * WP], f32, tag=f"mr58{tagsuffix}")
                nc.vector.memset(
                    mr58[:].rearrange("p (g c) -> p g c", c=WP)
                    [:, :, 0:WP:WP - 1], 0.0,
                )
                nc.vector.tensor_copy(
                    out=mr58[:].rearrange("p (g c) -> p g c", c=WP)[:, :, 1:57],
                    in_=mr[:].rearrange("p (g c) -> p g c", c=W),
                )
                mexp = maskp.tile([P, H * WP], fp8, tag=f"mexp{tagsuffix}")
                me3 = mexp[:].rearrange("p (r c) -> p r c", r=H)
                for hb in range(NCHUNK):
                    # image 0's casts go to ACT: the startup critical path
                    # is the serial DVE chain, and ACT is idle then
                    if cast_eng is nc.scalar:
                        nc.scalar.copy(
                            me3[:, hb * POOL_K:(hb + 1) * POOL_K, :],
                            mr58[:, hb * WP:(hb + 1) * WP]
                            .unsqueeze(1).broadcast_to((P, POOL_K, WP)),
                        )
                    else:
                        nc.vector.tensor_copy(
                            out=me3[:, hb * POOL_K:(hb + 1) * POOL_K, :],
                            in_=mr58[:, hb * WP:(hb + 1) * WP]
                            .unsqueeze(1).broadcast_to((P, POOL_K, WP)),
                        )
                return mexp

            mexp_h = expand_mask(m49h, "h")
            mexp_l = expand_mask(m49l, "l")

            # --- quantize: ACT rounds, DVE clamps + masked-stores ---
            # t tiles have one guard col each side; the store reads t at a
            # -1 column shift so its 58-wide rows stay contiguous.
            qh = qh_bufs[b % 2]
            ql = ql_bufs[b % 2]
            th = t_bufs[b % 2][0]
            tl = t_bufs[b % 2][1]
            bounds = [(i * H) // nsplit for i in range(nsplit + 1)]
            for ra, rb in zip(bounds, bounds[1:]):
                sl_ = slice(1 + ra * W, 1 + rb * W)
                for t, inv_s, qmax in ((th, inv_sh, 255.0), (tl, inv_sl, 15.0)):
                    nc.scalar.activation(
                        t[:, sl_], xt[:, ra * W:rb * W], Identity,
                        bias=magic_ap[:, 0:1], scale=inv_s,
                    )
                    nc.vector.tensor_scalar(
                        t[:, sl_], t[:, sl_], MAGIC, MAGIC + qmax,
                        op0=op.max, op1=op.min,
                    )
                nrows = rb - ra
                for t, mexp, tile_ap in ((th, mexp_h, qh), (tl, mexp_l, ql)):
                    # in0 rows overlap: 58 cols at row-stride 56 reads the
                    # pixel at (r, c-1); pad cols read neighbors * mask 0.
                    in0 = (
                        t[:, ra * W:rb * W]
                        .rearrange("p (r c) -> p r c", r=nrows)
                    )
                    in0.ap[2] = [1, WP]
                    nc.vector.scalar_tensor_tensor(
                        out=tile_ap[:, 59 + ra * WP:59 + rb * WP]
                        .rearrange("p (r c) -> p r c", r=nrows),
                        in0=in0,
                        scalar=MAGIC,
                        in1=mexp[:, ra * WP:rb * WP]
                        .rearrange("p (r c) -> p r c", r=nrows),
                        op0=op.subtract, op1=op.mult,
                    )
            return qh, ql

        def conv_chunk(b, qa, c):
            """One 464-wide window: 9 bf16 high taps + 3 DR + 3 single fp8
            low taps, all accumulating into one PSUM bank."""
            qh, ql = qa
            ps = conv_psum.tile([P, NW], f32, tag="ps", name=f"ps{b}_{c}")
            o = NW * c
            for t in range(NTAPS):
                nc.tensor.matmul(
                    ps[:], qwt_h[:, t * P:(t + 1) * P],
                    qh[:, o + HIGH_TAP_OFF[t]:o + HIGH_TAP_OFF[t] + NW],
                    start=(t == 0), stop=False,
                )
            for i, (_, _, roff, step) in enumerate(LOW_PAIRS):
                o0 = o + roff
                # overlapping AP: dim1 step pairs the window at taps
                # (delta, delta+step) out of the single activation plane
                rhs = (
                    ql[:, o0:o0 + NW].unsqueeze(1).broadcast_to((P, 2, NW))
                )
                rhs.ap[1] = [step, 2]
                lhsT = (
                    qwl_pairs[:, 2 * i * P:(2 * i + 2) * P]
                    .rearrange("p (two m) -> p two m", two=2)
                )
                nc.tensor.matmul(
                    ps[:], lhsT, rhs, start=False, stop=False, perf_mode=DR,
                )
            for s, soff in enumerate(LOW_SINGLE_OFF):
                o0 = o + soff
                nc.tensor.matmul(
                    ps[:], qwl_single[:, s * P:(s + 1) * P],
                    ql[:, o0:o0 + NW],
                    start=False, stop=(s == len(LOW_SINGLE_OFF) - 1),
                )
            return ps

        def evac_chunk(b, ps, c):
            seg = outs_pool.tile([P, H * POOL_K], f32, tag="seg")
            psv = ps[:].rearrange("p (r c) -> p r c", r=POOL_K)[:, :, 1:1 + W]
            nc.scalar.mul(seg[:], psv, sv["l"][:, 0:1])
            nc.sync.dma_start(
                out=y_d[b][:, c * H * POOL_K:(c + 1) * H * POOL_K], in_=seg[:]
            )

        # ---------------- schedule ----------------
        # software pipeline, two images deep: prep(b) for images 0 and 1 is
        # emitted up-front; during image b's convs we emit prep(b+2), whose
        # stores wait (WAR) on convs(b) releasing the shared buffer but
        # queue early so each engine's FIFO drains without head-of-line
        # blocking.
        fetch(0)
        fetch(1)
        qa = prep(0, nsplit=2)
        for b in range(bpc):
            fetch(b + 2)
            pss = []
            for c in range(NCHUNK):
                pss.append(conv_chunk(b, qa, c))
                if c < 3:
                    evac_chunk(b, pss[c], c)
            if b + 1 < bpc:
                qa = prep(b + 1)
            for c in range(3, NCHUNK):
                evac_chunk(b, pss[c], c)


def make_bass(inv_sh, inv_sl, c_svh, c_svl, bpc=BPC):
    import concourse.bacc as bacc
    import concourse.mybir as mybir
    from concourse.tile import TileContext

    f32 = mybir.dt.float32
    nc = bacc.Bacc("TRN2", debug=False)
    x = nc.dram_tensor("x", [bpc, P, NPIX], f32, kind="ExternalInput")
    wh = nc.dram_tensor("w_high", [P, P * NTAPS], f32, kind="ExternalInput")
    wl = nc.dram_tensor("w_low", [P, P * NTAPS], f32, kind="ExternalInput")
    y = nc.dram_tensor("y", [bpc, P, NPIX], f32, kind="ExternalOutput")
    aps = {"x": x.ap(), "w_high": wh.ap(), "w_low": wl.ap(), "y": y.ap()}
    with TileContext(nc) as tc:
        build_program(nc, tc, aps, inv_sh, inv_sl, c_svh, c_svl, bpc=bpc)
    nc.compile()
    return nc


def _scale_consts(act_scale_high, act_scale_low):
    sh = float(np.float32(act_scale_high))
    sl = float(np.float32(act_scale_low))
    inv_sh = float(np.float32(1.0 / np.float64(sh)))
    inv_sl = float(np.float32(1.0 / np.float64(sl)))
    c_svh = float(np.float32(np.float64(sh) / 127.0))
    c_svl = float(np.float32(np.float64(sl) / 7.0))
    return inv_sh, inv_sl, c_svh, c_svl


def _run(x, w_high, w_low, act_scale_high, act_scale_low, trace=False, **kw):
    from concourse import bass_utils

    x = np.ascontiguousarray(np.asarray(x, dtype=np.float32))
    w_high = np.ascontiguousarray(np.asarray(w_high, dtype=np.float32))
    w_low = np.ascontiguousarray(np.asarray(w_low, dtype=np.float32))

    inv_sh, inv_sl, c_svh, c_svl = _scale_consts(act_scale_high, act_scale_low)
    nc = make_bass(inv_sh, inv_sl, c_svh, c_svl)

    wh_flat = w_high.reshape(P, P * NTAPS)
    wl_flat = w_low.reshape(P, P * NTAPS)
    in_maps = []
    for core in range(N_CORES):
        xs = x[core * BPC:(core + 1) * BPC].reshape(BPC, P, NPIX)
        in_maps.append(
            {
                "x": np.ascontiguousarray(xs),
                "w_high": wh_flat,
                "w_low": wl_flat,
            }
        )
    res = bass_utils.run_bass_kernel_spmd(
        nc, in_maps, core_ids=list(range(N_CORES)), trace=trace, **kw
    )
    y = np.concatenate([r["y"].reshape(BPC, P, H, W) for r in res.results], axis=0)
    return y, res


def kernel(x, w_high, w_low, act_scale_high, act_scale_low):
    y, _ = _run(x, w_high, w_low, act_scale_high, act_scale_low)
    return y
